# revision 22
# baseline (speedup 1.0000x reference)
"""Multi-head self-attention Trainium2 kernel.

Problem: B=2, N=2048, D=1024, H=16 heads (HD=64), fp32 I/O.

Sharding (8 cores): core c handles batch b = c//4 and the 4-head group
g = c%4 (data parallel on B, tensor parallel on heads).  Each core:
  1. QKV projection for its 768 columns (q cols pre-scaled by HD^-0.5),
     producing qT/kT channel-major and V row-major augmented with a
     ones column.
  2. Transposed attention, two heads packed per pass (head A in PE rows
     0-63, head B in rows 64-127 -> concurrent row-group matmuls):
     S^T[m, n] scores in PSUM, exp per m-tile (split between ScalarE's
     LUT exp and a VectorE bit-trick exp -- i16 = round(s*128/ln2 +
     (127*128 - 5.5)) reinterpreted as bf16 -- to break the ScalarE
     throughput floor), PV matmul contracting over m with the ones
     column yielding the softmax denominator as row 64.
  3. Normalization: fast-approx reciprocal of the denominator row,
     broadcast across 64 partitions via a K=1 matmul, multiply.
  4. Output projection against its 256 rows of w_proj -> fp32 partial.
Host sums the 4 partials per batch and adds b_proj.

Schedule (v3): host orders w columns [kT p0 | qT p0 | kT p1 | qT p1 |
v] so the first quarter's weight gate is one 0.5 MB DMA; inputs stream
on the 3 HWDGE rings in big criticality-ordered instructions; dummy
K=1 matmuls warm the PE clock (HAM) during the DMA wait.  The
attention loop is a uniform software pipeline: scores(t), exp(t),
injected filler (V / qk chunks / epilogue+proj items in <=900ns
units), PV(t-1); each quarter's final PV drain + psum release carry
into the next quarter's first two iterations.  The tail runs the last
quarter's epilogue on parallel engines and projects in 512-col halves.
"""

import numpy as np
import ml_dtypes

B, N, D, H = 2, 2048, 1024, 16
HD = D // H  # 64
SCALE = HD ** -0.5
NCORES = 8
HPC = H // 4  # heads per core
CPC = HPC * HD  # channels per core = 256
P = 128
DT = D // P  # 8 contraction tiles
NT = N // P  # 16 sequence tiles

# VectorE bit-trick exp: bf16 bits = round(x * 128/ln2 + (127*128 - C)).
# C centers the linear-mantissa sawtooth (max rel err ~3%; attention's
# softmax-weighted sums average it far below the bf16 noise floor).
EXP_A = 128.0 / np.log(2.0)
EXP_B = 16256.0 - 5.5
DVE_MTS = (1, 3, 5, 7, 9, 11, 13, 15)  # m-tiles whose exp runs on VectorE

_CACHE = {}


def build_nc():
    import concourse.tile as tile
    from concourse import bacc, mybir

    nc = bacc.Bacc("TRN2", target_bir_lowering=False, debug=False,
                   num_devices=NCORES)
    bf16 = mybir.dt.bfloat16
    xt = nc.dram_tensor("xt", [D, N], bf16, kind="ExternalInput").ap()
    w = nc.dram_tensor("w", [D, 3 * CPC], bf16, kind="ExternalInput").ap()
    wp = nc.dram_tensor("wp", [CPC, D], bf16, kind="ExternalInput").ap()
    # bf16 partials halve the aggregate cross-core output traffic
    # (8 cores share one HBM link); the host sums them in fp32
    y = nc.dram_tensor("y", [N, D], bf16, kind="ExternalOutput").ap()

    with tile.TileContext(nc) as tc:
        _mha_tile_kernel(tc, y, xt, w, wp)
    nc.compile()
    return nc


def _mha_tile_kernel(tc, y, xt, w, wp):
    from contextlib import ExitStack
    from concourse import mybir

    nc = tc.nc
    bf16 = mybir.dt.bfloat16
    f32 = mybir.dt.float32
    i16 = mybir.dt.int16
    EXP = mybir.ActivationFunctionType.Exp
    MULT = mybir.AluOpType.mult
    ADD = mybir.AluOpType.add

    with ExitStack() as ctx:
        consts = ctx.enter_context(tc.tile_pool(name="consts", bufs=1))
        work = ctx.enter_context(tc.tile_pool(name="work", bufs=1))
        ebpool = ctx.enter_context(tc.tile_pool(name="eb", bufs=5))
        ypool = ctx.enter_context(tc.tile_pool(name="yp", bufs=4))
        rpool = ctx.enter_context(tc.tile_pool(name="rp", bufs=4))
        pvspool = ctx.enter_context(tc.tile_pool(name="pvs", bufs=6))
        ps_sc = ctx.enter_context(
            tc.tile_pool(name="ps_sc", bufs=2, space="PSUM"))   # 2x2 banks
        ps_pv = ctx.enter_context(
            tc.tile_pool(name="ps_pv", bufs=2, space="PSUM"))   # 2x1 banks
        ps_sm = ctx.enter_context(
            tc.tile_pool(name="ps_sm", bufs=1, space="PSUM"))   # 1x2 banks

        # ---- input DMAs: 3 HWDGE rings, criticality-ordered FIFOs.
        # Big multi-kt instructions (one InstDMACopy spreads over all 16
        # SDMA engines) amortize the per-instruction fixed cost; the
        # critical prefix (w[kT p0|qT p0] + xt half 0, 2.5 MB) lands at
        # the aggregate link rate, staggered so qk chunks start early.
        r0, r1, r2 = nc.sync, nc.scalar, nc.gpsimd

        w_sb = work.tile([P, DT, 3 * CPC], bf16, tag="w")
        xt_sb = work.tile([P, DT, N], bf16, tag="xt")
        wp_sb = work.tile([P, 2, D], bf16, tag="wp")

        def wload(eng, c0, c1):
            eng.dma_start(
                w_sb[:, :, c0:c1],
                w[:, c0:c1].rearrange("(k p) c -> p k c", p=P))

        def xload(eng, k0, k1, half):
            n0 = half * 1024
            eng.dma_start(
                xt_sb[:, k0:k1, n0:n0 + 1024],
                xt[k0 * P:k1 * P, n0:n0 + 1024].rearrange(
                    "(k p) n -> p k n", p=P))

        wload(r0, 0, 256)            # kT p0 + qT p0 (critical)
        xload(r1, 0, 3, 0)           # xt half 0 (critical)
        xload(r2, 3, 5, 0)
        xload(r0, 5, 8, 0)
        xload(r0, 0, 2, 1)           # xt half 1 (feeds kT p0 m>=1024)
        xload(r1, 4, 6, 1)
        wload(r2, 512, 768)          # v columns (feeds V groups)
        xload(r0, 2, 4, 1)
        xload(r1, 6, 8, 1)
        wload(r1, 256, 512)          # kT p1 + qT p1 (feeds aux)
        r2.dma_start(wp_sb, wp.rearrange("(c p) d -> p c d", p=P))

        ones_sb = consts.tile([1, N], bf16, tag="ones")
        nc.vector.memset(ones_sb, 1.0)

        qk_sb = work.tile([P, 4, N], bf16, tag="qk")
        vaug_sb = work.tile([P, NT, HPC, HD + 1], bf16, tag="vaug")
        nc.vector.memset(vaug_sb[:, :, :, HD:HD + 1], 1.0)
        outT_sb = work.tile([P, 2, N], bf16, tag="outT")

        # ---- PE warm-up: dummy K=1 matmuls on the ones row keep the
        # HAM clock gate at 8/8 through the DMA wait so the prologue's
        # real matmuls run at 2.4 GHz.
        warm = ps_sm.tile([P, 512], f32, tag="sm", name="warm")
        for i in range(16):
            nc.tensor.matmul(warm, lhsT=ones_sb[:, 0:P],
                             rhs=ones_sb[:, 0:512],
                             start=(i == 0), stop=False)

        # ---- emission helpers ----
        def qk_group_chunks(slot, wct, half, pool=None, copy_eng=None):
            """qT/kT channel-major: psum[c 128, n 1024] accumulated over
            d; copy to qk_sb slot as bf16.  Returned as ~0.9us chunks so
            injections never starve the exp engines."""
            wcol = wct * P
            n0 = half * 1024
            state = {}

            def emit_dts(dts, first, last):
                if not state:
                    p = pool if pool is not None else ps_sm
                    state["ps"] = p.tile([P, 1024], f32,
                                         tag="sc" if p is ps_sc else "sm",
                                         name=f"qk{slot}{half}")
                ps = state["ps"]
                for idx, dt in enumerate(dts):
                    for j in range(2):
                        nc.tensor.matmul(
                            ps[:, j * 512:(j + 1) * 512],
                            lhsT=w_sb[:, dt, wcol:wcol + P],
                            rhs=xt_sb[:, dt,
                                      n0 + j * 512:n0 + (j + 1) * 512],
                            start=(first and idx == 0),
                            stop=(last and idx == len(dts) - 1))
            def emit_copy():
                if copy_eng is nc.scalar:
                    nc.scalar.copy(out=qk_sb[:, slot, n0:n0 + 512],
                                   in_=state["ps"][:, 0:512])
                    nc.scalar.copy(out=qk_sb[:, slot, n0 + 512:n0 + 1024],
                                   in_=state["ps"][:, 512:1024])
                else:
                    nc.vector.tensor_copy(
                        out=qk_sb[:, slot, n0:n0 + 1024], in_=state["ps"])

            # the copy is its own item, popped one iteration after the
            # last chunk, so the copy engine's FIFO never head-of-line
            # blocks on the chunk matmuls
            return [lambda: emit_dts((0, 1), True, False),
                    lambda: emit_dts((2, 3), False, False),
                    lambda: emit_dts((4, 5), False, False),
                    lambda: emit_dts((6, 7), False, True),
                    emit_copy]

        def emit_v_mms(mt):
            """V row-major, all 4 heads: psum[m 128, c 256] over d."""
            c0 = 2 * CPC
            ps = ps_sm.tile([P, CPC], f32, tag="sm", name=f"v{mt}")
            for dt in range(DT):
                nc.tensor.matmul(
                    ps, lhsT=xt_sb[:, dt, mt * P:(mt + 1) * P],
                    rhs=w_sb[:, dt, c0:c0 + CPC],
                    start=(dt == 0), stop=(dt == DT - 1))
            return ps

        def emit_v_copy(mt, ps):
            """One iteration behind the matmuls so the DVE queue never
            head-of-line blocks on PE results."""
            nc.vector.tensor_copy(
                out=vaug_sb[:, mt, :, 0:HD],
                in_=ps.rearrange("p (h d) -> p h d", h=HPC))

        def emit_rec_head(pvs, rbfs, i):
            """All-DVE chain: hop the denominator row to base partition
            0 (the custom reciprocal op misreads nonzero base
            partitions), approx-reciprocal, cast to bf16."""
            dcp = rpool.tile([1, 512], f32, tag="dcp")
            nc.vector.tensor_copy(out=dcp, in_=pvs[i][HD:HD + 1, :])
            rec = rpool.tile([1, 512], f32, tag="rec")
            nc.vector.reciprocal_approx_fast(out=rec, in_=dcp)
            nc.vector.tensor_copy(out=rbfs[i], in_=rec)

        def emit_epi_bc(pair, q, pvs, rbfs, i, pending):
            """K=1 matmul broadcast of 1/denom; the multiply into outT
            runs on the next pop (keeps the DVE queue dependency-local)."""
            n0 = q * 512
            bp = i * HD
            bc = ps_sm.tile([HD, 512], f32, tag="sm",
                            name=f"bc{pair}{q}{i}")
            nc.tensor.matmul(bc, lhsT=ones_sb[:, 0:HD], rhs=rbfs[i],
                             start=True, stop=True)

            def mul():
                nc.vector.tensor_mul(
                    out=outT_sb[bp:bp + HD, pair, n0:n0 + 512],
                    in0=bc, in1=pvs[i][0:HD, :])
            pending.insert(0, ("eng", mul))

        proj_state = {}

        def emit_proj_half(nt, ec, pending, tail=False):
            """Output projection rows nt*128.., columns ec*512..: two
            matmuls now; half-copy + half-DMA on the next pop."""
            if ec == 0:
                pool = ps_sc if tail else ps_sm
                proj_state[nt] = (
                    pool.tile([P, 1024], f32, tag="sc" if tail else "sm",
                              name=f"pj{nt}"),
                    ypool.tile([P, D], bf16, tag="y", name=f"y{nt}"))
            ps, yt = proj_state[nt]
            for ct in range(2):
                nc.tensor.matmul(
                    ps[:, ec * 512:(ec + 1) * 512],
                    lhsT=outT_sb[:, ct, nt * P:(nt + 1) * P],
                    rhs=wp_sb[:, ct, ec * 512:(ec + 1) * 512],
                    start=(ct == 0), stop=(ct == 1))

            def copy_dma():
                if (nt + ec) % 2 == 0:
                    nc.scalar.copy(out=yt[:, ec * 512:(ec + 1) * 512],
                                   in_=ps[:, ec * 512:(ec + 1) * 512])
                else:
                    nc.vector.tensor_copy(
                        out=yt[:, ec * 512:(ec + 1) * 512],
                        in_=ps[:, ec * 512:(ec + 1) * 512])
                eng = nc.sync if (2 * nt + ec) % 2 == 0 else nc.gpsimd
                eng.dma_start(
                    y[nt * P:(nt + 1) * P, ec * 512:(ec + 1) * 512],
                    yt[:, ec * 512:(ec + 1) * 512])
            pending.insert(0, ("eng", copy_dma))

        score_ps = {}

        def emit_scores(t):
            """Scores for global step t -- emitted one iteration ahead
            so exp(t) is never gated behind same-iteration PE work."""
            pr, qq, mt = t // 64, (t % 64) // 16, t % 16
            n0 = qq * 512
            ps = ps_sc.tile([P, 1024], f32, tag="sc", name=f"s{t}")
            for i in range(2):
                bp = i * HD
                nc.tensor.matmul(
                    ps[:, i * 512:(i + 1) * 512],
                    lhsT=qk_sb[bp:bp + HD, 2 + pr, mt * P:(mt + 1) * P],
                    rhs=qk_sb[bp:bp + HD, pr, n0:n0 + 512],
                    start=True, stop=True)
            score_ps[t] = ps

        # ---- prologue: kT p0 / qT p0 half-0 groups, chunk-interleaved
        # so neither head-of-line blocks the other on late DMA parts ----
        kc = qk_group_chunks(2, 0, 0, copy_eng=nc.scalar)
        qc = qk_group_chunks(0, 1, 0, pool=ps_sc, copy_eng=nc.scalar)
        for a, b in zip(kc, qc):
            a()
            b()

        early = qk_group_chunks(2, 0, 1)   # kT p0, m 1024:2048
        # first two chunks ride the xt-half-1 DMA tail in the prologue
        # (they sit behind scores(0..1), so they never delay exp(0))
        aux = qk_group_chunks(0, 1, 1)     # qT p0 half 1 (needed q2)
        aux += qk_group_chunks(3, 2, 0)    # kT p1
        aux += qk_group_chunks(3, 2, 1)
        aux += qk_group_chunks(1, 3, 0)    # qT p1
        aux += qk_group_chunks(1, 3, 1)

        # ---- attention: uniform pipeline over (pair, quarter, m-tile).
        # scores run one iteration ahead; every engine queue's ops have
        # their inputs ready when they reach the queue head.
        emit_scores(0)
        emit_scores(1)
        early.pop(0)()
        early.pop(0)()
        pending = []   # <=0.9us PE items popped one per spare iteration
        carry = None   # previous quarter's (pv, eb, pair, pvs, rbfs, q)
        last = None
        vq = []        # deferred V-group copies
        for pair in range(2):
            for q in range(4):
                pv = [ps_pv.tile([HD + 1, 512], f32, tag="pv",
                                 name=f"pv{pair}{q}{i}") for i in range(2)]
                pvs = [pvspool.tile([HD + 1, 512], f32, tag="pvs",
                                    name=f"pvs{pair}{q}{i}")
                       for i in range(2)]
                rbfs = [rpool.tile([1, 512], bf16, tag="rbf",
                                   name=f"rbf{pair}{q}{i}")
                        for i in range(2)]
                for mt in range(NT):
                    it = q * NT + mt
                    t = pair * 64 + it
                    # 1) exp: ScalarE LUT or VectorE bit-trick
                    ps = score_ps.pop(t)
                    if mt in DVE_MTS:
                        ebi = ebpool.tile([P, 1024], i16, tag="ebi")
                        nc.vector.tensor_scalar(
                            out=ebi, in0=ps, scalar1=EXP_A,
                            scalar2=EXP_B, op0=MULT, op1=ADD)
                        eb = ebi.bitcast(bf16)
                    else:
                        eb = ebpool.tile([P, 1024], bf16, tag="eb")
                        nc.scalar.activation(out=eb, in_=ps, func=EXP)
                    # 2) carried work from the previous quarter
                    if mt == 0 and carry is not None:
                        while vq:          # V(15) copy before its PV
                            vq.pop(0)()
                        cpv, ceb, cpair = carry[0], carry[1], carry[2]
                        for i in range(2):
                            nc.tensor.matmul(
                                cpv[i],
                                lhsT=vaug_sb[:, NT - 1, 2 * cpair + i, :],
                                rhs=ceb[:, i * 512:(i + 1) * 512],
                                start=False, stop=True)
                    if mt == 1 and carry is not None:
                        cpv, cpvs, crbfs = carry[0], carry[3], carry[4]
                        cpair, cq = carry[2], carry[5]
                        for i in range(2):
                            nc.scalar.copy(out=cpvs[i], in_=cpv[i])
                        for i in range(2):
                            pending.append(
                                ("eng",
                                 lambda pvs_=cpvs, rbfs_=crbfs, i_=i:
                                 emit_rec_head(pvs_, rbfs_, i_)))
                        for i in range(2):
                            pending.append(
                                ("pe",
                                 lambda pr=cpair, qq=cq, pvs_=cpvs,
                                 rbfs_=crbfs, i_=i:
                                 emit_epi_bc(pr, qq, pvs_, rbfs_, i_,
                                             pending)))
                        if cpair == 1:
                            for nt2 in range(4 * cq, 4 * cq + 4):
                                for ec in range(2):
                                    pending.append(
                                        ("pe",
                                         lambda nt_=nt2, ec_=ec:
                                         emit_proj_half(nt_, ec_,
                                                        pending)))
                        carry = None
                    # 3) injected filler PE work
                    if pair == 0 and q == 0:
                        if vq:
                            vq.pop(0)()      # V(mt-1) copy (before PV)
                        vps = emit_v_mms(mt)
                        vq.append(lambda mt_=mt, ps_=vps:
                                  emit_v_copy(mt_, ps_))
                        if 2 < mt <= 6 and early:
                            early.pop(0)()   # kT p0 second m-half
                    elif pair == 0 and (it % 2 == 0 or it >= 60) and aux:
                        aux.pop(0)()
                    else:
                        # pop pendings: engine-only items until (and
                        # including) one PE item, at most 3 per step
                        npop = 0
                        while pending and npop < 3:
                            kind, fn = pending.pop(0)
                            fn()
                            npop += 1
                            if kind == "pe":
                                break
                    # 4) software-pipelined PV (one iteration behind)
                    if mt > 0:
                        for i in range(2):
                            nc.tensor.matmul(
                                pv[i],
                                lhsT=vaug_sb[:, mt - 1, 2 * pair + i, :],
                                rhs=eb_prev[:, i * 512:(i + 1) * 512],
                                start=(mt == 1), stop=False)
                    eb_prev = eb
                    # 5) scores for step t+2 (reuses exp(t)'s psum slot)
                    if t + 2 < 128:
                        emit_scores(t + 2)
                if pair == 1 and q == 3:
                    last = (pv, eb_prev, pair, pvs, rbfs, q)
                else:
                    carry = (pv, eb_prev, pair, pvs, rbfs, q)

        # ---- tail: final quarter's drain + epilogue + projection,
        # dependency chain spread across parallel engines ----
        pv, eb_last, pair, pvs, rbfs, q = last
        for i in range(2):
            nc.tensor.matmul(pv[i], lhsT=vaug_sb[:, NT - 1, 2 * pair + i, :],
                             rhs=eb_last[:, i * 512:(i + 1) * 512],
                             start=False, stop=True)
        # denominator rows read straight from PSUM on DVE while the
        # value rows release to SBUF on the idle ScalarE; bf16 casts on
        # ScalarE; dummy matmuls keep the PE clock warm through the
        # DVE/ACT latency chain.
        dcps = []
        for i in range(2):
            dcp = rpool.tile([1, 512], f32, tag="dcp")
            nc.vector.tensor_copy(out=dcp, in_=pv[i][HD:HD + 1, :])
            dcps.append(dcp)
        for i in range(2):
            nc.scalar.copy(out=pvs[i][0:HD, :], in_=pv[i][0:HD, :])
        for i in range(6):
            nc.tensor.matmul(warm, lhsT=ones_sb[:, 0:P],
                             rhs=ones_sb[:, 0:512],
                             start=(i == 0), stop=False)
        recs = []
        for i in range(2):
            rec = rpool.tile([1, 512], f32, tag="rec")
            nc.vector.reciprocal_approx_fast(out=rec, in_=dcps[i])
            recs.append(rec)
        for i in range(2):
            nc.scalar.copy(out=rbfs[i], in_=recs[i])
        while pending:
            pending.pop(0)[1]()
        for i in range(2):
            bp = i * HD
            bc = ps_sm.tile([HD, 512], f32, tag="sm", name=f"bct{i}")
            nc.tensor.matmul(bc, lhsT=ones_sb[:, 0:HD], rhs=rbfs[i],
                             start=True, stop=True)
            nc.vector.tensor_mul(
                out=outT_sb[bp:bp + HD, pair, q * 512:q * 512 + 512],
                in0=bc, in1=pvs[i][0:HD, :])
        for nt in range(4 * q, 4 * q + 4):
            for ec in range(2):
                emit_proj_half(nt, ec, pending, tail=True)
                pending.pop(0)[1]()   # its copy+DMA, immediately


def make_in_maps(x, w_qkv, b_qkv, w_proj):
    """Build the 8 per-core input dicts (host-side sharding).

    w columns are ordered [kT p0 | qT p0 | kT p1 | qT p1 | v] so the
    device's first-quarter weight gate is a single 0.5 MB DMA.  Biases
    are not sent to the device: b_k shifts every logit in a softmax row
    by the same amount (cancels exactly), b_v shifts the attention
    output by a constant (folded into y on the host as b_v @ w_proj),
    and b_q is zero for this problem (kernel() falls back to an exact
    host path if it ever is not).
    """
    bf = ml_dtypes.bfloat16
    x = np.asarray(x, np.float32)
    w_qkv = np.asarray(w_qkv, np.float32)
    w_proj = np.asarray(w_proj, np.float32)

    xts = [np.ascontiguousarray(x[b].T).astype(bf) for b in range(B)]
    w_augs = []
    wps = []
    for g in range(4):
        c0 = g * CPC
        wq = w_qkv[:, c0:c0 + CPC] * SCALE
        wk = w_qkv[:, D + c0:D + c0 + CPC]
        wv = w_qkv[:, 2 * D + c0:2 * D + c0 + CPC]
        w_slice = np.concatenate(
            [wk[:, 0:P], wq[:, 0:P], wk[:, P:CPC], wq[:, P:CPC], wv],
            axis=1).astype(bf)
        w_augs.append(np.ascontiguousarray(w_slice))
        wps.append(np.ascontiguousarray(w_proj[c0:c0 + CPC, :]).astype(bf))

    in_maps = []
    for core in range(NCORES):
        b, g = core // 4, core % 4
        in_maps.append({"xt": xts[b], "w": w_augs[g], "wp": wps[g]})
    return in_maps


def _host_reference(x, w_qkv, b_qkv, w_proj, b_proj):
    """Exact numpy fallback (used only if b_q is nonzero, which the
    problem's setup_inputs never produces)."""
    x = np.asarray(x, np.float32)
    qkv = x @ np.asarray(w_qkv, np.float32) + np.asarray(b_qkv, np.float32)
    qkv = qkv.reshape(B, N, 3, H, HD).transpose(2, 0, 3, 1, 4)
    q, k, v = qkv[0], qkv[1], qkv[2]
    att = np.einsum("bhnd,bhmd->bhnm", q, k) * SCALE
    att = np.exp(att - att.max(-1, keepdims=True))
    att /= att.sum(-1, keepdims=True)
    out = np.einsum("bhnm,bhmd->bhnd", att, v)
    out = out.transpose(0, 2, 1, 3).reshape(B, N, D)
    return out @ np.asarray(w_proj, np.float32) + np.asarray(b_proj,
                                                             np.float32)


def core_reference(in_map):
    """Numpy reference for ONE core's shard (for CoreSim verification)."""
    xt = np.asarray(in_map["xt"], np.float32)  # [D, N]
    w = np.asarray(in_map["w"], np.float32)    # [D, 768]
    wp = np.asarray(in_map["wp"], np.float32)  # [256, D]
    qkv = xt.T @ w                             # [N, 768]
    out = np.zeros((N, CPC), np.float32)
    for h in range(HPC):
        pr, idx = h // 2, h % 2
        k = qkv[:, pr * 2 * P + idx * HD:pr * 2 * P + idx * HD + HD]
        q = qkv[:, pr * 2 * P + P + idx * HD:pr * 2 * P + P + idx * HD + HD]
        v = qkv[:, 2 * CPC + h * HD:2 * CPC + (h + 1) * HD]
        s = q @ k.T  # scale already folded into wq
        p = np.exp(s - s.max(axis=-1, keepdims=True))
        p /= p.sum(axis=-1, keepdims=True)
        out[:, h * HD:(h + 1) * HD] = p @ v
    return out @ wp  # [N, D] partial


def kernel(x, w_qkv, b_qkv, w_proj, b_proj):
    from concourse.bass_utils import run_bass_kernel_spmd

    b_qkv = np.asarray(b_qkv, np.float32)
    if np.any(b_qkv[:D]):
        # nonzero q-bias does not cancel in softmax; exact host fallback
        # (never taken for this problem's setup_inputs)
        return _host_reference(x, w_qkv, b_qkv, w_proj, b_proj)

    in_maps = make_in_maps(x, w_qkv, b_qkv, w_proj)
    if "nc" not in _CACHE:
        _CACHE["nc"] = build_nc()
    res = run_bass_kernel_spmd(_CACHE["nc"], in_maps,
                               core_ids=list(range(NCORES)))
    outs = [np.asarray(r["y"], np.float32) for r in res.results]
    y = np.empty((B, N, D), np.float32)
    for b in range(B):
        y[b] = outs[4 * b] + outs[4 * b + 1] + outs[4 * b + 2] + outs[4 * b + 3]
    # bias: b_k cancels in softmax; b_v shifts attention output by a
    # constant -> y += b_v @ w_proj; plus the projection bias
    y += b_qkv[2 * D:] @ np.asarray(w_proj, np.float32)
    y += np.asarray(b_proj, np.float32)
    return y
